# revision 1
# baseline (speedup 1.0000x reference)
"""Trainium2 Bass kernel for a 3-layer GCN (nn_BaselineGCN).

Strategy (8 NeuronCores, node partitioning by dst):
  - Host: compute deg/dis, partition edges by owner of dst (6250 nodes/core,
    padded to 6272), sort by (dst-window, src-half), build int16 gather
    indices (full node table split into two 25088-row halves so indices fit
    int16) plus per-edge local-dst values for one-hot construction.
  - Device, per layer (bf16 tables, fp32 accumulation):
      * data-parallel matmul  Zs_own = dis ⊙ (H_own @ W)        (TensorE)
      * AllGather Zs chunks -> full 50176-row bf16 table in HBM (collective)
      * per 128-dst window: dma_gather source rows (memory-bound part),
        one-hot(dst_local) built on DVE via is_equal vs iota, segment-sum
        via PE matmul accumulation in PSUM, epilogue
        relu(dis ⊙ (acc + Zs_own_w) + b), TensorE transpose -> next H^T
        kept resident in SBUF.
      * self-loops handled analytically via the dis^2 self term.
  - Layer 3 (64 outputs) runs on a 128-wide bf16 table (upper 64 cols
    garbage, excluded by slicing) so the whole edge path is uniform bf16.
"""
import sys
import os

sys.path.insert(0, "/opt/trn_rl_repo")

import numpy as np

NC_CORES = 8
GMAX = 8  # max groups (=1024 indices) per dma_gather call
GATH_BUFS = 4  # gather-tile pool depth (first GATH_BUFS windows are memset)


def _cdiv(a, b):
    return (a + b - 1) // b


# ---------------------------------------------------------------------------
# Host-side preprocessing
# ---------------------------------------------------------------------------
def preprocess(edge_index, N):
    src = np.asarray(edge_index[0], dtype=np.int64)
    dst = np.asarray(edge_index[1], dtype=np.int64)
    deg = np.bincount(dst, minlength=N).astype(np.float32) + np.float32(1.0)
    dis = (np.float32(1.0) / np.sqrt(deg)).astype(np.float32)

    CH = N // NC_CORES
    NWIN = _cdiv(CH, 128)
    CHP = NWIN * 128
    # split each core's chunk into A (windows 0..NWA-1) and B (the rest) so
    # the AllGather of A can fire mid-phase; int16 gather indices address
    # each half-table separately.
    # A as large as int16 gather indices allow (NC*HA <= 32768) so the
    # exposed tail AllGather over B is as small as possible
    NWA = min(NWIN - 1, 32768 // (NC_CORES * 128)) if NWIN > 1 else NWIN
    HA = NWA * 128            # rows per core in table A
    HB = CHP - HA             # rows per core in table B (may be 0 if NWIN==1)
    src_c = src // CH         # owning core of each src node
    src_o = src % CH          # offset within core

    counts = np.zeros((NC_CORES, NWIN, 2), dtype=np.int64)
    percore = []
    for c in range(NC_CORES):
        sel = (dst >= c * CH) & (dst < (c + 1) * CH)
        sc, so = src_c[sel], src_o[sel]
        ed = dst[sel] - c * CH
        w = ed >> 7
        h = (so >= HA).astype(np.int64)
        eidx = np.where(h == 0, sc * HA + so, sc * HB + (so - HA))
        order = np.lexsort((ed, h, w))
        eidx, ed, w, h = eidx[order], ed[order], w[order], h[order]
        np.add.at(counts[c], (w, h), 1)
        percore.append((eidx, ed, w, h))

    G = _cdiv(counts, 128).max(axis=0)  # [NWIN, 2]

    import ml_dtypes

    # shared call schedule: per (window, half), gather calls of <=GMAX groups
    calls = []  # (wi, hi, g0, gc)
    for wi in range(NWIN):
        for hi in range(2):
            g0 = 0
            while g0 < G[wi, hi]:
                gc = min(GMAX, G[wi, hi] - g0)
                calls.append((wi, hi, g0, gc))
                g0 += gc

    cores = []
    for c in range(NC_CORES):
        eidx, ed, w, h = percore[c]
        idx_parts, dstl_parts = [], []
        pos = 0
        for wi in range(NWIN):
            for hi in range(2):
                n = counts[c, wi, hi]
                g = G[wi, hi]
                seg_idx = np.full(g * 128, -1, dtype=np.int16)
                seg_dstl = np.full(g * 128, 255.0, dtype=np.float32)
                if n:
                    seg_idx[:n] = eidx[pos:pos + n].astype(np.int16)
                    seg_dstl[:n] = (ed[pos:pos + n] - wi * 128).astype(np.float32)
                    pos += n
                idx_parts.append(seg_idx)
                dstl_parts.append(seg_dstl)
        idx_all = np.concatenate(idx_parts)
        dstl_all = np.concatenate(dstl_parts)
        TOT_G = len(idx_all) // 128

        # per-call valid counts; empty calls keep one dummy idx (0) because a
        # zero-valid gather is undefined
        seg_base = {}
        pos2 = 0
        for wi in range(NWIN):
            for hi in range(2):
                seg_base[(wi, hi)] = pos2
                pos2 += G[wi, hi] * 128
        ncounts = np.zeros(len(calls), dtype=np.int32)
        for k, (wi, hi, g0, gc) in enumerate(calls):
            n = int(counts[c, wi, hi])
            v = min(max(n - g0 * 128, 0), gc * 128)
            if v == 0:
                idx_all[seg_base[(wi, hi)] + g0 * 128] = 0
                v = 1
            ncounts[k] = v

        # device layouts
        idx_tiled = np.tile(idx_all.reshape(-1, 16).T, (8, 1)).copy()
        # dstl: [128 edge-slot, TOT_G] bf16
        dstl_tiled = np.ascontiguousarray(
            dstl_all.reshape(TOT_G, 128).T).astype(ml_dtypes.bfloat16)
        d = np.ones(CHP, np.float32)
        d[:CH] = dis[c * CH:(c + 1) * CH]
        dis_win = np.ascontiguousarray(d.reshape(NWIN, 128).T)
        cores.append(dict(idx=idx_tiled, dstl=dstl_tiled, dis_win=dis_win,
                          ncounts=ncounts))
    return dis, G, cores, CH, NWIN, CHP, NWA, len(calls)


# ---------------------------------------------------------------------------
# Bass program
# ---------------------------------------------------------------------------
def build_program(DIN, DRS, DTS, G, NWIN, CHP, NWA, TOT_IDX, TOT_G,
                  G_CAP, NCALLS, biases_nonzero):
    """DRS: real per-layer output dims [256,256,64];
    DTS: padded table dims [256,256,128]."""
    from concourse import bacc, bass, tile, mybir

    f32 = mybir.dt.float32
    bf16 = mybir.dt.bfloat16
    i16 = mybir.dt.int16
    ADD = mybir.AluOpType.add
    EQ = mybir.AluOpType.is_equal
    CPY = mybir.ActivationFunctionType.Copy
    NL = len(DRS)

    nc = bacc.Bacc("TRN2", target_bir_lowering=False, debug=False,
                   enable_asserts=False, num_devices=NC_CORES,
                   num_swdge_queues=4, dynamic_dma_scratch_size=32768)

    # --- I/O tensors ---
    xT_d = nc.dram_tensor("xT", [DIN, CHP], bf16, kind="ExternalInput")
    W_d = [nc.dram_tensor(f"W{i}", [DRS[i - 1] if i else DIN, DRS[i]], bf16,
                          kind="ExternalInput") for i in range(NL)]
    bias_d = [nc.dram_tensor(f"bias{i}", [128, DRS[i]], f32,
                             kind="ExternalInput") for i in range(NL)]
    idx_d = nc.dram_tensor("idx", [128, TOT_IDX // 16], i16, kind="ExternalInput")
    dstl_d = nc.dram_tensor("dstl", [128, TOT_G], bf16, kind="ExternalInput")
    iotag_d = nc.dram_tensor("iotag", [128, 128 * G_CAP], bf16,
                             kind="ExternalInput")
    dis_d = nc.dram_tensor("dis_win", [128, NWIN], f32, kind="ExternalInput")
    ident_d = nc.dram_tensor("ident", [128, 128], bf16, kind="ExternalInput")
    cnt_d = nc.dram_tensor("ncounts", [1, NCALLS], mybir.dt.int32,
                           kind="ExternalInput")
    out_d = nc.dram_tensor("out", [CHP, DRS[-1]], f32, kind="ExternalOutput")

    with tile.TileContext(nc) as tc:
        with (
            tc.tile_pool(name="const", bufs=1) as constp,
            tc.tile_pool(name="ht", bufs=1) as htp,
            tc.tile_pool(name="wts", bufs=2) as wtsp,
            tc.tile_pool(name="zs", bufs=3) as zsp,
            tc.tile_pool(name="gath", bufs=GATH_BUFS) as gathp,
            tc.tile_pool(name="oh", bufs=4) as ohp,
            tc.tile_pool(name="epi", bufs=3) as epip,
            tc.tile_pool(name="psz", bufs=2, space="PSUM") as pszp,
            tc.tile_pool(name="psw", bufs=3, space="PSUM") as pswp,
            tc.tile_pool(name="pst", bufs=2, space="PSUM") as pstp,
            tc.tile_pool(name="dram", bufs=1, space="DRAM") as dramp,
        ):
            # --- persistent SBUF constants ---
            idx_t = constp.tile([128, TOT_IDX // 16], i16, tag="idx")
            nc.sync.dma_start(idx_t[:], idx_d[:])
            dstl_t = constp.tile([128, TOT_G], bf16, tag="dstl")
            nc.sync.dma_start(dstl_t[:], dstl_d[:])
            iotag_t = constp.tile([128, 128 * G_CAP], bf16, tag="iotag")
            nc.sync.dma_start(iotag_t[:], iotag_d[:])
            dis_t = constp.tile([128, NWIN], f32, tag="dis")
            nc.sync.dma_start(dis_t[:], dis_d[:])
            ident_t = constp.tile([128, 128], bf16, tag="ident")
            nc.sync.dma_start(ident_t[:], ident_d[:])
            cnt_t = constp.tile([1, NCALLS], mybir.dt.int32, tag="cnt")
            nc.sync.dma_start(cnt_t[:], cnt_d[:])
            cnt_regs = [nc.gpsimd.alloc_register(f"gcnt{i}") for i in range(4)]
            bias_t = []
            for i in range(NL):
                if biases_nonzero[i]:
                    bt = constp.tile([128, DRS[i]], f32, tag=f"bias{i}")
                    nc.sync.dma_start(bt[:], bias_d[i][:])
                    bias_t.append(bt)
                else:
                    bias_t.append(None)

            # --- H^T SBUF-resident double buffer: [k][128, CHP] bf16 ---
            KT0 = DIN // 128
            ht_cur = [htp.tile([128, CHP], bf16, tag=f"htA{k}",
                               name=f"htA{k}") for k in range(KT0)]
            for k in range(KT0):
                nc.sync.dma_start(ht_cur[k][:], xT_d[k * 128:(k + 1) * 128, :])
            ht_nxt = [htp.tile([128, CHP], bf16, tag=f"htB{k}",
                               name=f"htB{k}") for k in range(KT0)]

            zs_own = [dramp.tile([CHP, DTS[i]], bf16, tag=f"zso{i}",
                                 name=f"zs_own{i}") for i in range(NL)]
            zs_own_f = [dramp.tile([CHP, DRS[i]], f32, tag=f"zsof{i}",
                                   name=f"zs_own_f{i}") for i in range(NL)]
            HA = NWA * 128
            HB = CHP - HA
            zs_fullA = [dramp.tile([NC_CORES * HA, DTS[i]], bf16,
                                   tag=f"zsfA{i}", addr_space="Shared",
                                   name=f"zs_fullA{i}") for i in range(NL)]
            zs_fullB = [dramp.tile([NC_CORES * HB, DTS[i]], bf16,
                                   tag=f"zsfB{i}", addr_space="Shared",
                                   name=f"zs_fullB{i}") for i in range(NL)]

            RG = [list(range(NC_CORES))]

            def emit_z_tile(li, t, lhs_tiles):
                """Z-matmul + scale for node-tile t of layer li."""
                Dr = DRS[li]
                Dt = DTS[li]
                KT = DIN // 128 if li == 0 else DRS[li - 1] // 128
                psz = pszp.tile([128, Dr], f32, tag="psz", name="psz")
                for k in range(KT):
                    nc.tensor.matmul(psz[:],
                                     lhs_tiles[k][:, t * 128:(t + 1) * 128],
                                     wk[li][k][:],
                                     start=(k == 0), stop=(k == KT - 1))
                zstf = zsp.tile([128, Dr], f32, tag="zstf", name="zstf")
                nc.scalar.activation(zstf[:], psz[:], CPY,
                                     scale=dis_t[:, t:t + 1])
                nc.sync.dma_start(zs_own_f[li][t * 128:(t + 1) * 128, :],
                                  zstf[:])
                zst = zsp.tile([128, Dt], bf16, tag="zst", name="zst")
                nc.scalar.activation(zst[:, :Dr], psz[:], CPY,
                                     scale=dis_t[:, t:t + 1])
                nc.sync.dma_start(zs_own[li][t * 128:(t + 1) * 128, :Dr],
                                  zst[:, :Dr])

            def emit_ag(li, half):
                if half == 0:
                    nc.gpsimd.collective_compute(
                        "AllGather", bass.mybir.AluOpType.bypass,
                        replica_groups=RG,
                        ins=[zs_own[li][:HA, :]],
                        outs=[zs_fullA[li].opt()])
                else:
                    nc.gpsimd.collective_compute(
                        "AllGather", bass.mybir.AluOpType.bypass,
                        replica_groups=RG,
                        ins=[zs_own[li][HA:, :]],
                        outs=[zs_fullB[li].opt()])

            # weight tiles for every layer, loaded up front (small)
            wk = []
            for li in range(NL):
                KT = DIN // 128 if li == 0 else DRS[li - 1] // 128
                wkl = []
                for k in range(KT):
                    wt_ = wtsp.tile([128, DRS[li]], bf16, tag=f"wk{li}_{k}",
                                    name=f"wk{li}_{k}")
                    nc.sync.dma_start(wt_[:], W_d[li][k * 128:(k + 1) * 128, :])
                    wkl.append(wt_)
                wk.append(wkl)

            # ---- layer-0 z-phase + split AllGather ----
            for t in range(NWIN):
                emit_z_tile(0, t, ht_cur)
                if t == NWA - 1:
                    emit_ag(0, 0)
            emit_ag(0, 1)

            # ---- main loop: gather/aggregate layer li; z + AG of li+1
            #      interleaved so the collectives hide under the gathers ----
            for li in range(NL):
                Dr = DRS[li]
                Dt = DTS[li]
                idx_off16 = 0
                g_off = 0
                qrr = 0
                call_i = 0
                for w in range(NWIN):
                    Gl, Gh = int(G[w, 0]), int(G[w, 1])
                    Gt = Gl + Gh
                    assert Gt > 0, "empty window unsupported"
                    wt = gathp.tile([128, G_CAP, Dt], bf16, tag="gather",
                                    name="wt")
                    if li == 0 and w < GATH_BUFS:
                        # first pool rotation: clear so slots skipped by
                        # short gathers never hold NaN bit patterns
                        nc.vector.memset(wt[:], 0.0)
                    for half, gcnt, gbase in ((0, Gl, 0), (1, Gh, Gl)):
                        tbl = zs_fullA[li] if half == 0 else zs_fullB[li]
                        g0 = 0
                        while g0 < gcnt:
                            gc = min(GMAX, gcnt - g0)
                            reg = cnt_regs[qrr % 4]
                            nc.gpsimd.reg_load(
                                reg, cnt_t[0:1, call_i:call_i + 1])
                            nc.gpsimd.dma_gather(
                                wt[:, gbase + g0:gbase + g0 + gc, :],
                                tbl[:],
                                idx_t[:, idx_off16:idx_off16 + gc * 8],
                                num_idxs=gc * 128,
                                num_idxs_reg=reg,
                                elem_size=Dt,
                                queue_num=qrr % 4,
                            )
                            qrr += 1
                            call_i += 1
                            idx_off16 += gc * 8
                            g0 += gc
                    # one-hot: oh[p, j, g] = (dstl[p, g] == j)
                    oh = ohp.tile([128, 128, Gt], bf16, tag="oh", name="oh")
                    nc.vector.tensor_tensor(
                        oh[:],
                        dstl_t[:, g_off:g_off + Gt].unsqueeze(1)
                            .broadcast_to((128, 128, Gt)),
                        iotag_t[:].rearrange("p (j g) -> p j g", g=G_CAP)
                            [:, :, :Gt],
                        op=EQ,
                    )
                    g_off += Gt
                    psw = pswp.tile([128, Dt], f32, tag="psw", name="psw")
                    for g in range(Gt):
                        nc.tensor.matmul(psw[:], oh[:, :, g], wt[:, g, :],
                                         start=(g == 0), stop=(g == Gt - 1))
                    # epilogue: t1 = psw + zs_own_w (self term), fp32
                    zw = zsp.tile([128, Dr], f32, tag="zw", name="zw")
                    nc.sync.dma_start(
                        zw[:], zs_own_f[li][w * 128:(w + 1) * 128, :])
                    t1 = epip.tile([128, Dr], f32, tag="t1", name="t1")
                    nc.vector.tensor_tensor(t1[:], psw[:, :Dr], zw[:], op=ADD)
                    if bias_t[li] is not None:
                        nc.vector.tensor_tensor(t1[:], t1[:], bias_t[li][:],
                                                op=ADD)
                    if li < NL - 1:
                        h2 = epip.tile([128, Dr], bf16, tag="h2", name="h2")
                        nc.scalar.activation(
                            h2[:], t1[:],
                            bass.mybir.ActivationFunctionType.Relu,
                            scale=dis_t[:, w:w + 1])
                        for k in range(Dr // 128):
                            pst = pstp.tile([128, 128], bf16, tag="pst",
                                            name="pst")
                            nc.tensor.transpose(
                                pst[:], h2[:, k * 128:(k + 1) * 128],
                                ident_t[:])
                            nc.vector.tensor_copy(
                                ht_nxt[k][:, w * 128:(w + 1) * 128], pst[:])
                        # interleaved z for layer li+1 (its H^T tile-w is
                        # ready now); AG halves fire mid-phase
                        emit_z_tile(li + 1, w, ht_nxt)
                        if w == NWA - 1:
                            emit_ag(li + 1, 0)
                    else:
                        h2o = epip.tile([128, Dr], f32, tag="h2o", name="h2o")
                        nc.scalar.activation(h2o[:], t1[:], CPY,
                                             scale=dis_t[:, w:w + 1])
                        nc.sync.dma_start(out_d[w * 128:(w + 1) * 128, :],
                                          h2o[:])
                if li < NL - 1:
                    emit_ag(li + 1, 1)
                    ht_cur, ht_nxt = ht_nxt, ht_cur
    nc.compile()
    return nc


# ---------------------------------------------------------------------------
# Entry point
# ---------------------------------------------------------------------------
def kernel(x, edge_index, W1, b1, W2, b2, W3, b3):
    from concourse.bass_utils import run_bass_kernel_spmd
    import ml_dtypes

    bfnp = ml_dtypes.bfloat16
    x = np.asarray(x, dtype=np.float32)
    Ws = [np.asarray(w, dtype=np.float32) for w in (W1, W2, W3)]
    bs = [np.asarray(b, dtype=np.float32) for b in (b1, b2, b3)]

    N, DIN = x.shape
    DRS = [w.shape[1] for w in Ws]
    DTS = [max(d, 128) for d in DRS]
    NL = 3

    dis, G, cores, CH, NWIN, CHP, NWA, NCALLS = preprocess(edge_index, N)
    TOT_IDX = cores[0]["idx"].shape[1] * 16
    TOT_G = cores[0]["dstl"].shape[1]
    G_CAP = int((G[:, 0] + G[:, 1]).max())
    biases_nonzero = [bool(np.any(b != 0)) for b in bs]

    nc = build_program(DIN, DRS, DTS, G, NWIN, CHP, NWA, TOT_IDX, TOT_G,
                       G_CAP, NCALLS, biases_nonzero)

    ident = np.eye(128, dtype=bfnp)
    # iotag[p, j*G_CAP + g] = j
    iotag = np.tile(np.repeat(np.arange(128), G_CAP).astype(bfnp), (128, 1))
    in_maps = []
    for c in range(NC_CORES):
        xT = np.zeros((DIN, CHP), bfnp)
        xT[:, :CH] = x[c * CH:(c + 1) * CH].T.astype(bfnp)
        m = {
            "xT": xT,
            "idx": cores[c]["idx"],
            "dstl": cores[c]["dstl"],
            "iotag": iotag,
            "dis_win": cores[c]["dis_win"],
            "ident": ident,
            "ncounts": cores[c]["ncounts"][None, :],
        }
        for i in range(NL):
            m[f"W{i}"] = Ws[i].astype(bfnp)
            m[f"bias{i}"] = np.tile(bs[i][None, :], (128, 1))
        in_maps.append(m)

    trace = bool(int(os.environ.get("GCN_TRACE", "0")))
    res = run_bass_kernel_spmd(nc, in_maps, core_ids=list(range(NC_CORES)),
                               trace=trace)
    kernel.last_results = res
    out = np.concatenate([res.results[c]["out"][:CH] for c in range(NC_CORES)],
                         axis=0)
    return out.astype(np.float32)



# revision 3
# speedup vs baseline: 1.1129x; 1.1129x over previous
"""Trainium2 Bass kernel for a 3-layer GCN (nn_BaselineGCN).

Aggregate-first formulation (uses D~(HW) = (D~H)W):
  out_l = sigma( (D~ H_l) W_l + b_l ),  D~ = D^{-1/2}(A+I)D^{-1/2}

  - The gather table for layer l is T_l = dis (.) H_l (scaled by the SOURCE
    node's dis), 256-wide bf16 for every layer (512B gather descriptors).
  - Layer 1's table is dis (.) x == computable on the HOST: no z-phase, no
    layer-1 AllGather, gathers start at t~0.
  - Self-loop term: dis[i]*H[i] == T_own[i]; folded into the PSUM
    accumulation as one identity matmul per window.
  - Per dst-window epilogue: u = dis[w] (.) psw  (bf16), transpose via PE,
    u @ W_l, then T_{l+1}[w] = relu(dis[w] (.) (uW)) written to HBM and
    AllGathered (split A/B so the collective hides under the gathers).

Sharding: nodes partitioned across 8 cores by dst (6250/core, padded 6272);
edges sorted by (dst-window, src-half); int16 gather indices into two table
halves (A: 8*4096 rows, B: 8*2176 rows) so indices fit int16.
"""
import sys
import os

sys.path.insert(0, "/opt/trn_rl_repo")

import numpy as np

NC_CORES = 8
GMAX = 8  # max groups (=1024 indices) per dma_gather call
GATH_BUFS = 6  # gather-tile pool depth (first GATH_BUFS windows are memset)
D = 256  # feature width of every gather table


def _cdiv(a, b):
    return (a + b - 1) // b


# ---------------------------------------------------------------------------
# Host-side preprocessing (same edge partitioning as before; indices are
# shared by all three layers)
# ---------------------------------------------------------------------------
def preprocess(edge_index, N):
    src = np.asarray(edge_index[0], dtype=np.int64)
    dst = np.asarray(edge_index[1], dtype=np.int64)
    deg = np.bincount(dst, minlength=N).astype(np.float32) + np.float32(1.0)
    dis = (np.float32(1.0) / np.sqrt(deg)).astype(np.float32)

    CH = N // NC_CORES
    NWIN = _cdiv(CH, 128)
    CHP = NWIN * 128
    # A as large as int16 gather indices allow (NC*HA <= 32768); B the rest
    NWA = min(NWIN - 1, 32768 // (NC_CORES * 128)) if NWIN > 1 else NWIN
    HA = NWA * 128
    HB = CHP - HA
    src_c = src // CH
    src_o = src % CH

    counts = np.zeros((NC_CORES, NWIN, 2), dtype=np.int64)
    percore = []
    for c in range(NC_CORES):
        sel = (dst >= c * CH) & (dst < (c + 1) * CH)
        sc, so = src_c[sel], src_o[sel]
        ed = dst[sel] - c * CH
        w = ed >> 7
        h = (so >= HA).astype(np.int64)
        eidx = np.where(h == 0, sc * HA + so, sc * HB + (so - HA))
        order = np.lexsort((ed, h, w))
        eidx, ed, w, h = eidx[order], ed[order], w[order], h[order]
        np.add.at(counts[c], (w, h), 1)
        percore.append((eidx, ed, w, h))

    G = _cdiv(counts, 128).max(axis=0)  # [NWIN, 2]

    import ml_dtypes

    # shared call schedule: per (window, half), gather calls of <=GMAX groups
    calls = []  # (wi, hi, g0, gc)
    for wi in range(NWIN):
        for hi in range(2):
            g0 = 0
            while g0 < G[wi, hi]:
                gc = min(GMAX, G[wi, hi] - g0)
                calls.append((wi, hi, g0, gc))
                g0 += gc

    cores = []
    for c in range(NC_CORES):
        eidx, ed, w, h = percore[c]
        idx_parts, dstl_parts = [], []
        pos = 0
        for wi in range(NWIN):
            for hi in range(2):
                n = counts[c, wi, hi]
                g = G[wi, hi]
                seg_idx = np.full(g * 128, -1, dtype=np.int16)
                seg_dstl = np.full(g * 128, 255.0, dtype=np.float32)
                if n:
                    seg_idx[:n] = eidx[pos:pos + n].astype(np.int16)
                    seg_dstl[:n] = (ed[pos:pos + n] - wi * 128).astype(np.float32)
                    pos += n
                idx_parts.append(seg_idx)
                dstl_parts.append(seg_dstl)
        idx_all = np.concatenate(idx_parts)
        dstl_all = np.concatenate(dstl_parts)
        TOT_G = len(idx_all) // 128

        # per-call valid counts; empty calls keep one dummy idx (0) because a
        # zero-valid gather is undefined
        seg_base = {}
        pos2 = 0
        for wi in range(NWIN):
            for hi in range(2):
                seg_base[(wi, hi)] = pos2
                pos2 += G[wi, hi] * 128
        ncounts = np.zeros(len(calls), dtype=np.int32)
        for k, (wi, hi, g0, gc) in enumerate(calls):
            n = int(counts[c, wi, hi])
            v = min(max(n - g0 * 128, 0), gc * 128)
            if v == 0:
                idx_all[seg_base[(wi, hi)] + g0 * 128] = 0
                v = 1
            ncounts[k] = v

        # device layouts
        idx_tiled = np.tile(idx_all.reshape(-1, 16).T, (8, 1)).copy()
        dstl_tiled = np.ascontiguousarray(
            dstl_all.reshape(TOT_G, 128).T).astype(ml_dtypes.bfloat16)
        d = np.ones(CHP, np.float32)
        d[:CH] = dis[c * CH:(c + 1) * CH]
        dis_win = np.ascontiguousarray(d.reshape(NWIN, 128).T)
        cores.append(dict(idx=idx_tiled, dstl=dstl_tiled, dis_win=dis_win,
                          ncounts=ncounts))
    return dis, G, cores, CH, NWIN, CHP, NWA, len(calls)


# ---------------------------------------------------------------------------
# Bass program
# ---------------------------------------------------------------------------
def build_program(DRS, G, NWIN, CHP, NWA, TOT_IDX, TOT_G, G_CAP, NCALLS,
                  biases_nonzero):
    """DRS: per-layer output dims [256, 256, 64]; every gather table is
    D=256 wide."""
    from concourse import bacc, bass, tile, mybir

    f32 = mybir.dt.float32
    bf16 = mybir.dt.bfloat16
    i16 = mybir.dt.int16
    ADD = mybir.AluOpType.add
    EQ = mybir.AluOpType.is_equal
    CPY = mybir.ActivationFunctionType.Copy
    RELU = mybir.ActivationFunctionType.Relu
    NL = len(DRS)
    HA = NWA * 128
    HB = CHP - HA

    nc = bacc.Bacc("TRN2", target_bir_lowering=False, debug=False,
                   enable_asserts=False, num_devices=NC_CORES,
                   num_swdge_queues=4, dynamic_dma_scratch_size=32768)

    # --- I/O tensors ---
    xsA_d = nc.dram_tensor("xsA", [NC_CORES * HA, D], bf16,
                           kind="ExternalInput")
    xsB_d = nc.dram_tensor("xsB", [NC_CORES * HB, D], bf16,
                           kind="ExternalInput")
    xso_d = nc.dram_tensor("xso", [CHP, D], bf16, kind="ExternalInput")
    W_d = [nc.dram_tensor(f"W{i}", [D, DRS[i]], bf16, kind="ExternalInput")
           for i in range(NL)]
    bias_d = [nc.dram_tensor(f"bias{i}", [128, DRS[i]], f32,
                             kind="ExternalInput") for i in range(NL)]
    idx_d = nc.dram_tensor("idx", [128, TOT_IDX // 16], i16,
                           kind="ExternalInput")
    dstl_d = nc.dram_tensor("dstl", [128, TOT_G], bf16, kind="ExternalInput")
    iotag_d = nc.dram_tensor("iotag", [128, 128 * G_CAP], bf16,
                             kind="ExternalInput")
    dis_d = nc.dram_tensor("dis_win", [128, NWIN], f32, kind="ExternalInput")
    ident_d = nc.dram_tensor("ident", [128, 128], bf16, kind="ExternalInput")
    cnt_d = nc.dram_tensor("ncounts", [1, NCALLS], mybir.dt.int32,
                           kind="ExternalInput")
    out_d = nc.dram_tensor("out", [CHP, DRS[-1]], f32, kind="ExternalOutput")

    with tile.TileContext(nc) as tc:
        with (
            tc.tile_pool(name="const", bufs=1) as constp,
            tc.tile_pool(name="wts", bufs=2) as wtsp,
            tc.tile_pool(name="town", bufs=3) as townp,
            tc.tile_pool(name="gath", bufs=GATH_BUFS) as gathp,
            tc.tile_pool(name="oh", bufs=4) as ohp,
            tc.tile_pool(name="epi", bufs=3) as epip,
            tc.tile_pool(name="ht", bufs=4) as htp,
            tc.tile_pool(name="psw", bufs=2, space="PSUM") as pswp,
            tc.tile_pool(name="ps2", bufs=2, space="PSUM") as ps2p,
            tc.tile_pool(name="pst", bufs=2, space="PSUM") as pstp,
            tc.tile_pool(name="dram", bufs=1, space="DRAM") as dramp,
        ):
            # --- persistent SBUF constants ---
            idx_t = constp.tile([128, TOT_IDX // 16], i16, tag="idx")
            nc.sync.dma_start(idx_t[:], idx_d[:])
            dstl_t = constp.tile([128, TOT_G], bf16, tag="dstl")
            nc.sync.dma_start(dstl_t[:], dstl_d[:])
            iotag_t = constp.tile([128, 128 * G_CAP], bf16, tag="iotag")
            nc.sync.dma_start(iotag_t[:], iotag_d[:])
            dis_t = constp.tile([128, NWIN], f32, tag="dis")
            nc.sync.dma_start(dis_t[:], dis_d[:])
            ident_t = constp.tile([128, 128], bf16, tag="ident")
            nc.sync.dma_start(ident_t[:], ident_d[:])
            cnt_t = constp.tile([1, NCALLS], mybir.dt.int32, tag="cnt")
            nc.sync.dma_start(cnt_t[:], cnt_d[:])
            cnt_regs = [nc.gpsimd.alloc_register(f"gcnt{i}") for i in range(4)]
            bias_t = []
            for i in range(NL):
                if biases_nonzero[i]:
                    bt = constp.tile([128, DRS[i]], f32, tag=f"bias{i}")
                    nc.sync.dma_start(bt[:], bias_d[i][:])
                    bias_t.append(bt)
                else:
                    bias_t.append(None)

            # weight tiles (k-major, 2 tiles of [128, DRS[l]] each)
            wk = []
            for li in range(NL):
                wkl = []
                for k in range(D // 128):
                    wt_ = wtsp.tile([128, DRS[li]], bf16, tag=f"wk{li}_{k}",
                                    name=f"wk{li}_{k}")
                    nc.sync.dma_start(wt_[:], W_d[li][k * 128:(k + 1) * 128, :])
                    wkl.append(wt_)
                wk.append(wkl)

            # next-layer tables (own chunk + AllGathered full halves)
            t_own = [dramp.tile([CHP, D], bf16, tag=f"town{i}",
                                name=f"t_own{i}") for i in range(NL - 1)]
            t_fullA = [dramp.tile([NC_CORES * HA, D], bf16, tag=f"tfA{i}",
                                  addr_space="Shared", name=f"t_fullA{i}")
                       for i in range(NL - 1)]
            t_fullB = [dramp.tile([NC_CORES * HB, D], bf16, tag=f"tfB{i}",
                                  addr_space="Shared", name=f"t_fullB{i}")
                       for i in range(NL - 1)]

            RG = [list(range(NC_CORES))]

            def emit_ag(li, half):
                # AllGather own table chunk (li: produced-by layer index)
                if half == 0:
                    nc.gpsimd.collective_compute(
                        "AllGather", bass.mybir.AluOpType.bypass,
                        replica_groups=RG,
                        ins=[t_own[li][:HA, :]],
                        outs=[t_fullA[li].opt()])
                else:
                    nc.gpsimd.collective_compute(
                        "AllGather", bass.mybir.AluOpType.bypass,
                        replica_groups=RG,
                        ins=[t_own[li][HA:, :]],
                        outs=[t_fullB[li].opt()])

            for li in range(NL):
                Dr = DRS[li]
                tblA = xsA_d if li == 0 else t_fullA[li - 1]
                tblB = xsB_d if li == 0 else t_fullB[li - 1]
                selft = xso_d if li == 0 else t_own[li - 1]
                idx_off16 = 0
                g_off = 0
                qrr = 0
                call_i = 0
                for w in range(NWIN):
                    Gl, Gh = int(G[w, 0]), int(G[w, 1])
                    Gt = Gl + Gh
                    assert Gt > 0, "empty window unsupported"
                    wt = gathp.tile([128, G_CAP, D], bf16, tag="gather",
                                    name="wt")
                    if li * NWIN + w < GATH_BUFS:
                        # first pool rotation: clear so slots skipped by
                        # short gathers never hold NaN bit patterns
                        nc.vector.memset(wt[:], 0.0)
                    for half, gcnt, gbase in ((0, Gl, 0), (1, Gh, Gl)):
                        tbl = tblA if half == 0 else tblB
                        g0 = 0
                        while g0 < gcnt:
                            gc = min(GMAX, gcnt - g0)
                            reg = cnt_regs[qrr % 4]
                            nc.gpsimd.reg_load(
                                reg, cnt_t[0:1, call_i:call_i + 1])
                            nc.gpsimd.dma_gather(
                                wt[:, gbase + g0:gbase + g0 + gc, :],
                                tbl[:],
                                idx_t[:, idx_off16:idx_off16 + gc * 8],
                                num_idxs=gc * 128,
                                num_idxs_reg=reg,
                                elem_size=D,
                                queue_num=qrr % 4,
                            )
                            qrr += 1
                            call_i += 1
                            idx_off16 += gc * 8
                            g0 += gc
                    # one-hot: oh[p, j, g] = (dstl[p, g] == j)
                    oh = ohp.tile([128, 128, Gt], bf16, tag="oh", name="oh")
                    nc.vector.tensor_tensor(
                        oh[:],
                        dstl_t[:, g_off:g_off + Gt].unsqueeze(1)
                            .broadcast_to((128, 128, Gt)),
                        iotag_t[:].rearrange("p (j g) -> p j g", g=G_CAP)
                            [:, :, :Gt],
                        op=EQ,
                    )
                    g_off += Gt
                    # self term streamed through the same PSUM accumulation
                    town = townp.tile([128, D], bf16, tag="town", name="town")
                    nc.sync.dma_start(town[:],
                                      selft[w * 128:(w + 1) * 128, :])
                    psw = pswp.tile([128, D], f32, tag="psw", name="psw")
                    for g in range(Gt):
                        nc.tensor.matmul(psw[:], oh[:, :, g], wt[:, g, :],
                                         start=(g == 0), stop=False)
                    nc.tensor.matmul(psw[:], ident_t[:], town[:],
                                     start=False, stop=True)
                    # u = dis[w] (.) psw  (bf16) -> transpose -> @ W_li
                    h2 = epip.tile([128, D], bf16, tag="h2", name="h2")
                    nc.scalar.activation(h2[:], psw[:], CPY,
                                         scale=dis_t[:, w:w + 1])
                    p2 = ps2p.tile([128, Dr], f32, tag="p2", name="p2")
                    for k in range(D // 128):
                        pst = pstp.tile([128, 128], bf16, tag="pst",
                                        name="pst")
                        nc.tensor.transpose(
                            pst[:], h2[:, k * 128:(k + 1) * 128], ident_t[:])
                        hT = htp.tile([128, 128], bf16, tag="hT", name="hT")
                        nc.vector.tensor_copy(hT[:], pst[:])
                        nc.tensor.matmul(p2[:], hT[:], wk[li][k][:],
                                         start=(k == 0),
                                         stop=(k == D // 128 - 1))
                    if li < NL - 1:
                        if bias_t[li] is not None:
                            tb = epip.tile([128, Dr], f32, tag="tb",
                                           name="tb")
                            nc.vector.tensor_tensor(tb[:], p2[:],
                                                    bias_t[li][:], op=ADD)
                            src_ap = tb
                        else:
                            src_ap = p2
                        t2 = epip.tile([128, Dr], bf16, tag="t2", name="t2")
                        nc.scalar.activation(t2[:], src_ap[:], RELU,
                                             scale=dis_t[:, w:w + 1])
                        nc.sync.dma_start(t_own[li][w * 128:(w + 1) * 128, :],
                                          t2[:])
                        if w == NWA - 1:
                            emit_ag(li, 0)
                    else:
                        if bias_t[li] is not None:
                            tb = epip.tile([128, Dr], f32, tag="tb",
                                           name="tb")
                            nc.vector.tensor_tensor(tb[:], p2[:],
                                                    bias_t[li][:], op=ADD)
                            src_ap = tb
                        else:
                            src_ap = p2
                        ot = epip.tile([128, Dr], f32, tag="ot", name="ot")
                        nc.scalar.activation(ot[:], src_ap[:], CPY)
                        nc.sync.dma_start(out_d[w * 128:(w + 1) * 128, :],
                                          ot[:])
                if li < NL - 1:
                    emit_ag(li, 1)
    nc.compile()
    return nc


# ---------------------------------------------------------------------------
# Entry point
# ---------------------------------------------------------------------------
def kernel(x, edge_index, W1, b1, W2, b2, W3, b3):
    from concourse.bass_utils import run_bass_kernel_spmd
    import ml_dtypes

    bfnp = ml_dtypes.bfloat16
    x = np.asarray(x, dtype=np.float32)
    Ws = [np.asarray(w, dtype=np.float32) for w in (W1, W2, W3)]
    bs = [np.asarray(b, dtype=np.float32) for b in (b1, b2, b3)]

    N, DIN = x.shape
    assert DIN == D
    DRS = [w.shape[1] for w in Ws]
    NL = 3

    dis, G, cores, CH, NWIN, CHP, NWA, NCALLS = preprocess(edge_index, N)
    HA = NWA * 128
    HB = CHP - HA
    TOT_IDX = cores[0]["idx"].shape[1] * 16
    TOT_G = cores[0]["dstl"].shape[1]
    G_CAP = int((G[:, 0] + G[:, 1]).max())
    biases_nonzero = [bool(np.any(b != 0)) for b in bs]

    nc = build_program(DRS, G, NWIN, CHP, NWA, TOT_IDX, TOT_G, G_CAP, NCALLS,
                       biases_nonzero)

    # host-side layer-1 table: xs = dis (.) x, packed into A/B halves
    xs = (dis[:, None] * x).astype(bfnp)
    xsA = np.zeros((NC_CORES * HA, D), bfnp)
    xsB = np.zeros((NC_CORES * HB, D), bfnp)
    for c in range(NC_CORES):
        na = min(HA, CH)
        xsA[c * HA:c * HA + na] = xs[c * CH:c * CH + na]
        nb = CH - na
        if nb > 0:
            xsB[c * HB:c * HB + nb] = xs[c * CH + na:(c + 1) * CH]

    ident = np.eye(128, dtype=bfnp)
    # iotag[p, j*G_CAP + g] = j
    iotag = np.tile(np.repeat(np.arange(128), G_CAP).astype(bfnp), (128, 1))
    in_maps = []
    for c in range(NC_CORES):
        xso = np.zeros((CHP, D), bfnp)
        xso[:CH] = xs[c * CH:(c + 1) * CH]
        m = {
            "xsA": xsA,
            "xsB": xsB,
            "xso": xso,
            "idx": cores[c]["idx"],
            "dstl": cores[c]["dstl"],
            "iotag": iotag,
            "dis_win": cores[c]["dis_win"],
            "ident": ident,
            "ncounts": cores[c]["ncounts"][None, :],
        }
        for i in range(NL):
            m[f"W{i}"] = Ws[i].astype(bfnp)
            m[f"bias{i}"] = np.tile(bs[i][None, :], (128, 1))
        in_maps.append(m)

    trace = bool(int(os.environ.get("GCN_TRACE", "0")))
    res = run_bass_kernel_spmd(nc, in_maps, core_ids=list(range(NC_CORES)),
                               trace=trace)
    kernel.last_results = res
    out = np.concatenate([res.results[c]["out"][:CH] for c in range(NC_CORES)],
                         axis=0)
    return out.astype(np.float32)
